# revision 2
# baseline (speedup 1.0000x reference)
"""Trainium2 Bass kernel v5.4 for nn_MHParallelAttention (B=4,S=1024,H=16,DK=64).

Sharding: 8 cores = (batch) x (query-row half); core owns rows [b, s0:s0+512, :].

Key ideas on top of the fp16 baseline:
  * heads sorted by |Wc| on host (einsum over h is permutation-invariant);
    the 8 smallest-|Wc| heads (chunks 4-7) run their score contraction in
    fp8e4 DoubleRow mode (0.5 cyc/col): scores 32768 -> 20480 PE cycles,
    CoreSim rel-err 1.3e-2 vs the 2e-2 gate.
  * mask folded in BEFORE exp via an identity-weight matmul adding a
    maskneg (-60000) f16 tensor into each score psum group; exp then
    produces masked outputs directly and accum_out yields the row sums,
    killing the DVE mask/sum chain in the tail.
  * ALL input DMAs ride the sync/HWDGE queue in exact consumption order:
    HWDGE generation (~625ns/DMA) stays ahead of the serial DMA bus
    (~728ns/2KB-chunk), so arrival order is deterministic and the SWDGE
    path (1us/trigger of Pool time, uncontrollable start) is avoided.
  * 2-slot proj psum rotation + t0/t1 accumulators in-era; the PE emission
    interleaves score chunks so no kproj is ever queued behind a
    DVE-gated score matmul; t2/t3 fill the slots freed by tk6/tk7.
  * k0 split so the tanh stream starts ~3.8us; t3 accumulated half-first
    so the final exp is 2x[128,512] and the last chain is short.
"""

import os
import sys

import numpy as np

for _p in ("/opt/trn_rl_repo", "/root/.axon_site/_ro/trn_rl_repo"):
    if os.path.isdir(_p) and _p not in sys.path:
        sys.path.insert(0, _p)

import concourse.bass as bass
import concourse.mybir as mybir
import concourse.tile as tile
from concourse import bacc
from concourse.bass import ds, ts

H, DK = 16, 64
B, S = 4, 1024
SQ = 512
NCORES = 8
NJ = 8

F16 = mybir.dt.float16
F32 = mybir.dt.float32
F8 = mybir.dt.float8e4

Tanh = mybir.ActivationFunctionType.Tanh
Exp = mybir.ActivationFunctionType.Exp
Add = mybir.AluOpType.add
DR = mybir.MatmulPerfMode.DoubleRow

NEGM = -60000.0
PRE_COLS = 7562


def build_nc():
    nc = bacc.Bacc(None, target_bir_lowering=False, debug=False)

    # pre: host-packed prefix, sliced into slim HWDGE DMAs in consumption
    # order: [wkb 128 | biases 10 | k0h0 512 | wqb 128 | eye 128 | k0h1 512 |
    #         q01 1024 | k1 1024 | q23 1024 | k2 1024 | q45 1024 | q67 1024]
    pre = nc.dram_tensor("pre", [128, PRE_COLS], F16, kind="ExternalInput")
    kT = nc.dram_tensor("kT", [5, 128, S], F16, kind="ExternalInput")
    msk = nc.dram_tensor("msk", [4, 128, S], F16, kind="ExternalInput")
    out = nc.dram_tensor("out", [SQ, S], F16, kind="ExternalOutput")

    with tile.TileContext(nc) as tc:
        with (
            tc.tile_pool(name="const", bufs=1) as cst,
            tc.tile_pool(name="kin", bufs=1) as kin,
            tc.tile_pool(name="qin", bufs=1) as qin,
            tc.tile_pool(name="kpp", bufs=1) as kpp,
            tc.tile_pool(name="qpp", bufs=1) as qpp,
            tc.tile_pool(name="mrow", bufs=1) as mrp,
            tc.tile_pool(name="soft", bufs=1) as softp,
            tc.tile_pool(name="stat", bufs=16) as statp,
            tc.tile_pool(name="obuf", bufs=4) as obp,
            tc.tile_pool(name="rot", bufs=2, space="PSUM") as rotp,
            tc.tile_pool(name="pa0", bufs=1, space="PSUM") as p0p,
            tc.tile_pool(name="pa1", bufs=1, space="PSUM") as p1p,
        ):
            bootA = cst.tile([128, 650], F16, tag="bootA")
            bootB = cst.tile([128, 768], F16, tag="bootB")
            prim = cst.tile([128, 1], F32, tag="prim")
            nc.vector.memset(prim[:], 0.0)
            nc.scalar.activation(prim[:], prim[:], Tanh)
            wkb = bootA[:, ds(0, 128)]
            bkb = bootA[:, ds(128, 1)]
            bqb = bootA[:, ds(129, 1)]
            wcb16 = bootA[:, ds(130, NJ)]
            k0h0 = bootA[:, ds(138, 512)]
            wcb_t = cst.tile([128, NJ], F32, tag="wcb32")
            wcb = wcb_t[:]
            wqb = bootB[:, ds(0, 128)]
            eye = bootB[:, ds(128, 128)]
            k0h1 = bootB[:, ds(256, 512)]

            q01t = qin.tile([128, 1024], F16, tag="q01")
            q23t = qin.tile([128, 1024], F16, tag="q23")
            q45t = qin.tile([128, 1024], F16, tag="q45")
            q67t = qin.tile([128, 1024], F16, tag="q67")
            k1t = kin.tile([128, S], F16, tag="k1")
            k2t = kin.tile([128, S], F16, tag="k2")
            kch = [kin.tile([128, 1, S], F16, tag=f"k{j}", name=f"k{j}")
                   for j in range(3, NJ)]

            def kt(j):
                if j == 1:
                    return k1t[:]
                if j == 2:
                    return k2t[:]
                return kch[j - 3][:, 0, :]

            def qt(j):
                if j < 2:
                    return q01t[:, ds((j % 2) * 512, 512)]
                if j < 4:
                    return q23t[:, ds((j % 2) * 512, 512)]
                if j < 6:
                    return q45t[:, ds((j % 2) * 512, 512)]
                return q67t[:, ds((j % 2) * 512, 512)]

            mk = [mrp.tile([128, S], F16, tag=f"mk{t}", name=f"mk{t}")
                  for t in range(4)]

            # ---- ALL input DMAs on sync, in consumption order
            nc.sync.dma_start(out=bootA[:], in_=pre[:, ds(0, 650)])
            nc.vector.tensor_copy(wcb_t[:], wcb16)
            nc.sync.dma_start(out=bootB[:], in_=pre[:, ds(650, 768)])
            nc.sync.dma_start(out=q01t[:], in_=pre[:, ds(1418, 1024)])
            nc.sync.dma_start(out=k1t[:], in_=pre[:, ds(2442, 1024)])
            nc.sync.dma_start(out=q23t[:], in_=pre[:, ds(3466, 1024)])
            nc.sync.dma_start(out=k2t[:], in_=pre[:, ds(4490, 1024)])
            nc.sync.dma_start(out=q45t[:], in_=pre[:, ds(5514, 1024)])
            nc.sync.dma_start(out=q67t[:], in_=pre[:, ds(6538, 1024)])
            for j in range(3, NJ):
                nc.sync.dma_start(
                    out=kch[j - 3][:],
                    in_=kT[ds(j - 3, 1)].rearrange("j p k -> p j k"))
            for t in range(4):
                nc.sync.dma_start(
                    out=mk[t][:], in_=msk[ds(t, 1)].rearrange("t p k -> p t k"))

            # f16 kp for chunks 0-3; fp8 pair tiles for chunks 4-7
            kp = [kpp.tile([128, S], F16, tag=f"kp{j}", name=f"kp{j}")
                  for j in range(4)]
            kp8 = [kpp.tile([128, 2, S], F8, tag=f"kp8{a}", name=f"kp8{a}")
                   for a in range(2)]
            qp = [qpp.tile([128, SQ], F16, tag=f"qp{j}", name=f"qp{j}")
                  for j in range(4)]
            qp8 = [qpp.tile([128, 2, SQ], F8, tag=f"qp8{a}", name=f"qp8{a}")
                   for a in range(2)]

            def kp_out(j):
                if j < 4:
                    return kp[j][:]
                return kp8[(j - 4) // 2][:, (j - 4) % 2, :]

            # ---- proj ops into the 2-slot rotation
            def kproj0():
                pk = rotp.tile([128, S], F32, tag="rot", name="pk0")
                nc.tensor.matmul(pk[:, ds(0, 512)], wkb, k0h0)
                nc.scalar.activation(kp[0][:, ds(0, 512)], pk[:, ds(0, 512)],
                                     Tanh, bias=bkb)
                nc.tensor.matmul(pk[:, ds(512, 512)], wkb, k0h1)
                nc.scalar.activation(kp[0][:, ds(512, 512)],
                                     pk[:, ds(512, 512)], Tanh, bias=bkb)

            def kproj(j):
                pk = rotp.tile([128, S], F32, tag="rot", name=f"pk{j}")
                nc.tensor.matmul(pk[:, ds(0, 512)], wkb, kt(j)[:, ds(0, 512)])
                nc.tensor.matmul(pk[:, ds(512, 512)], wkb,
                                 kt(j)[:, ds(512, 512)])
                nc.scalar.activation(kp_out(j), pk[:], Tanh, bias=bkb)

            def qpair(p):
                pq = rotp.tile([128, S], F32, tag="rot", name=f"pq{p}")
                tq = statp.tile([128, S], F16, tag="tq", name=f"tq{p}", bufs=2)
                nc.tensor.matmul(pq[:, ds(0, 512)], wqb, qt(2 * p))
                nc.tensor.matmul(pq[:, ds(512, 512)], wqb, qt(2 * p + 1))
                nc.scalar.activation(tq[:], pq[:], Tanh, bias=bqb)
                for j in (2 * p, 2 * p + 1):
                    dst = (qp[j][:] if j < 4
                           else qp8[(j - 4) // 2][:, (j - 4) % 2, :])
                    nc.vector.tensor_scalar_mul(
                        dst, tq[:, ds((j % 2) * 512, 512)], wcb[:, ds(j, 1)])

            acc0 = p0p.tile([128, S], F32, tag="acc0", name="acc0")
            acc1 = p1p.tile([128, S], F32, tag="acc1", name="acc1")

            def f16_mm(pst, j, t, half, start):
                nc.tensor.matmul(pst[:, ds(half * 512, 512)],
                                 qp[j][:, ts(t, 128)],
                                 kp[j][:, ds(half * 512, 512)],
                                 start=start, stop=False)

            def dr_mm(pst, a, t, half, stop):
                nc.tensor.matmul(pst[:, ds(half * 512, 512)],
                                 qp8[a][:, :, ts(t, 128)],
                                 kp8[a][:, :, ds(half * 512, 512)],
                                 start=False, stop=stop, perf_mode=DR)

            def mask_mm(pst, t, half):
                nc.tensor.matmul(pst[:, ds(half * 512, 512)], eye,
                                 mk[t][:, ds(half * 512, 512)],
                                 start=False, stop=False)

            def s01(j, t_sel=(0, 1)):
                for t in t_sel:
                    a = acc0 if t == 0 else acc1
                    for half in range(2):
                        f16_mm(a, j, t, half, j == 0)

            # ---- era: stream = tk0ab,tq0,tk1,tq1,tk2,tq2,tk3,tq3,tk4..tk7.
            # Emission never queues a kproj behind a score mm that waits on
            # a LATER tanh/DVE product.
            kproj0()
            qpair(0)
            kproj(1)
            qpair(1)
            s01(0, (0,))
            kproj(2)
            s01(0, (1,))
            s01(1, (0,))
            qpair(2)
            s01(1, (1,))
            kproj(3)
            s01(2, (0,))
            qpair(3)
            s01(2, (1,))
            kproj(4)
            s01(3, (0,))
            kproj(5)
            s01(3, (1,))
            kproj(6)
            kproj(7)
            # DR pair A for t0/t1 (ready with tk5), mask01 while tk6/7 run
            for t, a in ((0, acc0), (1, acc1)):
                for half in range(2):
                    dr_mm(a, 0, t, half, False)
            for t, a in ((0, acc0), (1, acc1)):
                for half in range(2):
                    mask_mm(a, t, half)

            # ---- post-stream
            p2 = rotp.tile([128, S], F32, tag="rot", name="p2")
            p3 = rotp.tile([128, S], F32, tag="rot", name="p3")

            # t2 f16 chunks 0,1 in slot A (free after tk6-read)
            for half in range(2):
                f16_mm(p2, 0, 2, half, True)
                f16_mm(p2, 1, 2, half, False)
            # t0/t1 stops ASAP after tk7
            for t, a in ((0, acc0), (1, acc1)):
                for half in range(2):
                    dr_mm(a, 1, t, half, True)
            # interleave t2 and t3 (t3 half0 first so its exp can split)
            for half in range(2):
                f16_mm(p2, 2, 2, half, False)
                f16_mm(p2, 3, 2, half, False)
            f16_mm(p3, 0, 3, 0, True)
            mask_mm(p3, 3, 0)
            f16_mm(p3, 1, 3, 0, False)
            for half in range(2):
                mask_mm(p2, 2, half)
                dr_mm(p2, 0, 2, half, False)
            for half in range(2):
                dr_mm(p2, 1, 2, half, True)
            f16_mm(p3, 2, 3, 0, False)
            f16_mm(p3, 3, 3, 0, False)
            dr_mm(p3, 0, 3, 0, False)
            dr_mm(p3, 1, 3, 0, True)
            f16_mm(p3, 0, 3, 1, True)
            mask_mm(p3, 3, 1)
            for j in range(1, 4):
                f16_mm(p3, j, 3, 1, False)
            dr_mm(p3, 0, 3, 1, False)
            dr_mm(p3, 1, 3, 1, True)

            # ---- exp (+accum row sums) -> normalize -> out, per block
            ex = [softp.tile([128, S], F16, tag=f"ex{t}", name=f"ex{t}")
                  for t in range(4)]

            def finish(t, src, mul_eng):
                ssum = statp.tile([128, 1], F32, tag="ssum", name=f"ssum{t}")
                nc.scalar.activation(ex[t][:], src, Exp, accum_out=ssum[:])
                rec = statp.tile([128, 1], F32, tag="rec", name=f"rec{t}")
                nc.vector.reciprocal(rec[:], ssum[:])
                ot = obp.tile([128, S], F16, tag="ot", name=f"ot{t}")
                mul_eng.tensor_scalar_mul(ot[:], ex[t][:], rec[:])
                nc.sync.dma_start(out=out[ts(t, 128), :], in_=ot[:])

            finish(0, acc0[:], nc.vector)
            finish(1, acc1[:], nc.vector)
            finish(2, p2[:], nc.vector)
            finish(3, p3[:], nc.vector)

    nc.compile()
    return nc


_NC = None


def _get_nc():
    global _NC
    if _NC is None:
        _NC = build_nc()
    return _NC


def make_in_maps(query, key, mask, Wq, bq, Wk, bk, Wc, bc):
    query = np.asarray(query, np.float32)
    key = np.asarray(key, np.float32)
    mask = np.asarray(mask)
    Wq = np.asarray(Wq, np.float32)
    Wk = np.asarray(Wk, np.float32)
    Wc = np.asarray(Wc, np.float32)
    bq = np.asarray(bq, np.float32)
    bk = np.asarray(bk, np.float32)

    # sort heads by |Wc| descending: chunks 0-3 (8 largest) stay fp16,
    # chunks 4-7 (8 smallest) run fp8 DoubleRow
    order = np.argsort(-np.abs(Wc[0]))
    wc_s = Wc[0][order]

    def blockdiag(W):
        blk = np.zeros((128, 128), np.float32)
        blk[0:64, 0:64] = W.T
        blk[64:128, 64:128] = W.T
        return blk

    wk128 = blockdiag(Wk).astype(np.float16)
    wq128 = blockdiag(Wq).astype(np.float16)
    eye = np.eye(128, dtype=np.float16)
    wb16 = np.zeros((128, 10), np.float16)
    wb16[:, 0] = np.tile(bk.reshape(-1), 2)
    wb16[:, 1] = np.tile(bq.reshape(-1), 2)
    for j in range(NJ):
        wb16[0:64, 2 + j] = wc_s[2 * j]
        wb16[64:128, 2 + j] = wc_s[2 * j + 1]

    in_maps = []
    for c in range(NCORES):
        b, half = divmod(c, 2)
        s0 = half * SQ
        qh = query[b].reshape(H, S, DK)[order][:, s0:s0 + SQ, :]
        qTc = np.ascontiguousarray(
            qh.transpose(0, 2, 1)).reshape(NJ, 128, SQ).astype(np.float16)
        kh_ = key[b].reshape(H, S, DK)[order]
        kTc = np.ascontiguousarray(
            kh_.transpose(0, 2, 1)).reshape(NJ, 128, S).astype(np.float16)
        pre = np.concatenate(
            [wk128, wb16, kTc[0][:, 0:512], wq128, eye, kTc[0][:, 512:1024],
             qTc[0], qTc[1], kTc[1], qTc[2], qTc[3], kTc[2],
             qTc[4], qTc[5], qTc[6], qTc[7]], axis=1)
        assert pre.shape[1] == PRE_COLS, pre.shape
        mc = np.where(mask[b, s0:s0 + SQ, :] == 0, NEGM, 0.0)
        mc = np.ascontiguousarray(mc.reshape(4, 128, S)).astype(np.float16)
        in_maps.append({"pre": pre, "kT": kTc[3:], "msk": mc})
    return in_maps


def kernel(query, key, mask, Wq, bq, Wk, bk, Wc, bc):
    from concourse.bass_utils import run_bass_kernel_spmd

    nc = _get_nc()
    in_maps = make_in_maps(query, key, mask, Wq, bq, Wk, bk, Wc, bc)
    res = run_bass_kernel_spmd(nc, in_maps, list(range(NCORES)))
    full = np.empty((B, S, S), np.float32)
    for c in range(NCORES):
        b, half = divmod(c, 2)
        full[b, half * SQ:(half + 1) * SQ, :] = \
            res.results[c]["out"].astype(np.float32)
    return full


# revision 4
# speedup vs baseline: 1.0128x; 1.0128x over previous
"""Trainium2 Bass kernel v5.4 for nn_MHParallelAttention (B=4,S=1024,H=16,DK=64).

Sharding: 8 cores = (batch) x (query-row half); core owns rows [b, s0:s0+512, :].

Key ideas on top of the fp16 baseline:
  * heads sorted by |Wc| on host (einsum over h is permutation-invariant);
    the 8 smallest-|Wc| heads (chunks 4-7) run their score contraction in
    fp8e4 DoubleRow mode (0.5 cyc/col): scores 32768 -> 20480 PE cycles,
    CoreSim rel-err 1.3e-2 vs the 2e-2 gate.
  * mask folded in BEFORE exp via an identity-weight matmul adding a
    maskneg (-60000) f16 tensor into each score psum group; exp then
    produces masked outputs directly and accum_out yields the row sums,
    killing the DVE mask/sum chain in the tail.
  * ALL input DMAs ride the sync/HWDGE queue in exact consumption order:
    HWDGE generation (~625ns/DMA) stays ahead of the serial DMA bus
    (~728ns/2KB-chunk), so arrival order is deterministic and the SWDGE
    path (1us/trigger of Pool time, uncontrollable start) is avoided.
  * 2-slot proj psum rotation + t0/t1 accumulators in-era; the PE emission
    interleaves score chunks so no kproj is ever queued behind a
    DVE-gated score matmul; t2/t3 fill the slots freed by tk6/tk7.
  * k0 split so the tanh stream starts ~3.8us; t3 accumulated half-first
    so the final exp is 2x[128,512] and the last chain is short.
"""

import os
import sys

import numpy as np

for _p in ("/opt/trn_rl_repo", "/root/.axon_site/_ro/trn_rl_repo"):
    if os.path.isdir(_p) and _p not in sys.path:
        sys.path.insert(0, _p)

import concourse.bass as bass
import concourse.mybir as mybir
import concourse.tile as tile
from concourse import bacc
from concourse.bass import ds, ts

H, DK = 16, 64
B, S = 4, 1024
SQ = 512
NCORES = 8
NJ = 8

F16 = mybir.dt.float16
F32 = mybir.dt.float32
F8 = mybir.dt.float8e4

Tanh = mybir.ActivationFunctionType.Tanh
Exp = mybir.ActivationFunctionType.Exp
Add = mybir.AluOpType.add
DR = mybir.MatmulPerfMode.DoubleRow

NEGM = -60000.0
PRE_COLS = 7562


def build_nc():
    nc = bacc.Bacc(None, target_bir_lowering=False, debug=False)

    # pre: host-packed prefix, sliced into slim HWDGE DMAs in consumption
    # order: [wkb 128 | biases 10 | k0h0 512 | wqb 128 | eye 128 | k0h1 512 |
    #         q01 1024 | k1 1024 | q23 1024 | k2 1024 | q45 1024 | q67 1024]
    pre = nc.dram_tensor("pre", [128, PRE_COLS], F16, kind="ExternalInput")
    kT = nc.dram_tensor("kT", [5, 128, S], F16, kind="ExternalInput")
    msk = nc.dram_tensor("msk", [4, 128, S], F16, kind="ExternalInput")
    out = nc.dram_tensor("out", [SQ, S], F16, kind="ExternalOutput")

    with tile.TileContext(nc) as tc:
        with (
            tc.tile_pool(name="const", bufs=1) as cst,
            tc.tile_pool(name="kin", bufs=1) as kin,
            tc.tile_pool(name="qin", bufs=1) as qin,
            tc.tile_pool(name="kpp", bufs=1) as kpp,
            tc.tile_pool(name="qpp", bufs=1) as qpp,
            tc.tile_pool(name="mrow", bufs=1) as mrp,
            tc.tile_pool(name="soft", bufs=1) as softp,
            tc.tile_pool(name="stat", bufs=16) as statp,
            tc.tile_pool(name="obuf", bufs=4) as obp,
            tc.tile_pool(name="rot", bufs=2, space="PSUM") as rotp,
            tc.tile_pool(name="pa0", bufs=1, space="PSUM") as p0p,
            tc.tile_pool(name="pa1", bufs=1, space="PSUM") as p1p,
        ):
            bootA = cst.tile([128, 650], F16, tag="bootA")
            bootB = cst.tile([128, 768], F16, tag="bootB")
            prim = cst.tile([128, 1], F32, tag="prim")
            nc.vector.memset(prim[:], 0.0)
            nc.scalar.activation(prim[:], prim[:], Tanh)
            wkb = bootA[:, ds(0, 128)]
            bkb = bootA[:, ds(128, 1)]
            bqb = bootA[:, ds(129, 1)]
            wcb16 = bootA[:, ds(130, NJ)]
            k0h0 = bootA[:, ds(138, 512)]
            wcb_t = cst.tile([128, NJ], F32, tag="wcb32")
            wcb = wcb_t[:]
            wqb = bootB[:, ds(0, 128)]
            eye = bootB[:, ds(128, 128)]
            k0h1 = bootB[:, ds(256, 512)]

            q01t = qin.tile([128, 1024], F16, tag="q01")
            q23t = qin.tile([128, 1024], F16, tag="q23")
            q45t = qin.tile([128, 1024], F16, tag="q45")
            q67t = qin.tile([128, 1024], F16, tag="q67")
            k1t = kin.tile([128, S], F16, tag="k1")
            k2t = kin.tile([128, S], F16, tag="k2")
            kch = [kin.tile([128, 1, S], F16, tag=f"k{j}", name=f"k{j}")
                   for j in range(3, NJ)]

            def kt(j):
                if j == 1:
                    return k1t[:]
                if j == 2:
                    return k2t[:]
                return kch[j - 3][:, 0, :]

            def qt(j):
                if j < 2:
                    return q01t[:, ds((j % 2) * 512, 512)]
                if j < 4:
                    return q23t[:, ds((j % 2) * 512, 512)]
                if j < 6:
                    return q45t[:, ds((j % 2) * 512, 512)]
                return q67t[:, ds((j % 2) * 512, 512)]

            mk = [mrp.tile([128, S], F16, tag=f"mk{t}", name=f"mk{t}")
                  for t in range(4)]

            # ---- ALL input DMAs on sync, in consumption order
            nc.sync.dma_start(out=bootA[:], in_=pre[:, ds(0, 650)])
            nc.vector.tensor_copy(wcb_t[:], wcb16)
            nc.sync.dma_start(out=bootB[:], in_=pre[:, ds(650, 768)])
            nc.sync.dma_start(out=q01t[:], in_=pre[:, ds(1418, 1024)])
            nc.sync.dma_start(out=k1t[:], in_=pre[:, ds(2442, 1024)])
            nc.sync.dma_start(out=q23t[:], in_=pre[:, ds(3466, 1024)])
            nc.sync.dma_start(out=k2t[:], in_=pre[:, ds(4490, 1024)])
            nc.sync.dma_start(out=q45t[:], in_=pre[:, ds(5514, 1024)])
            nc.sync.dma_start(out=q67t[:], in_=pre[:, ds(6538, 1024)])
            for j in range(3, NJ):
                nc.sync.dma_start(
                    out=kch[j - 3][:],
                    in_=kT[ds(j - 3, 1)].rearrange("j p k -> p j k"))
            for t in range(4):
                nc.sync.dma_start(
                    out=mk[t][:], in_=msk[ds(t, 1)].rearrange("t p k -> p t k"))

            # f16 kp for chunks 0-3; fp8 pair tiles for chunks 4-7
            kp = [kpp.tile([128, S], F16, tag=f"kp{j}", name=f"kp{j}")
                  for j in range(4)]
            kp8 = [kpp.tile([128, 2, S], F8, tag=f"kp8{a}", name=f"kp8{a}")
                   for a in range(2)]
            qp = [qpp.tile([128, SQ], F16, tag=f"qp{j}", name=f"qp{j}")
                  for j in range(4)]
            qp8 = [qpp.tile([128, 2, SQ], F8, tag=f"qp8{a}", name=f"qp8{a}")
                   for a in range(2)]

            def kp_out(j):
                if j < 4:
                    return kp[j][:]
                return kp8[(j - 4) // 2][:, (j - 4) % 2, :]

            # ---- proj ops into the 2-slot rotation
            def kproj0():
                pk = rotp.tile([128, S], F32, tag="rot", name="pk0")
                nc.tensor.matmul(pk[:, ds(0, 512)], wkb, k0h0)
                nc.tensor.matmul(pk[:, ds(512, 512)], wkb, k0h1)
                nc.scalar.activation(kp[0][:], pk[:], Tanh, bias=bkb)

            def kproj(j):
                pk = rotp.tile([128, S], F32, tag="rot", name=f"pk{j}")
                nc.tensor.matmul(pk[:, ds(0, 512)], wkb, kt(j)[:, ds(0, 512)])
                nc.tensor.matmul(pk[:, ds(512, 512)], wkb,
                                 kt(j)[:, ds(512, 512)])
                nc.scalar.activation(kp_out(j), pk[:], Tanh, bias=bkb)

            def qpair(p):
                pq = rotp.tile([128, S], F32, tag="rot", name=f"pq{p}")
                tq = statp.tile([128, S], F16, tag="tq", name=f"tq{p}", bufs=2)
                nc.tensor.matmul(pq[:, ds(0, 512)], wqb, qt(2 * p))
                nc.tensor.matmul(pq[:, ds(512, 512)], wqb, qt(2 * p + 1))
                nc.scalar.activation(tq[:], pq[:], Tanh, bias=bqb)
                for j in (2 * p, 2 * p + 1):
                    dst = (qp[j][:] if j < 4
                           else qp8[(j - 4) // 2][:, (j - 4) % 2, :])
                    nc.vector.tensor_scalar_mul(
                        dst, tq[:, ds((j % 2) * 512, 512)], wcb[:, ds(j, 1)])

            acc0 = p0p.tile([128, S], F32, tag="acc0", name="acc0")
            acc1 = p1p.tile([128, S], F32, tag="acc1", name="acc1")

            def f16_mm(pst, j, t, half, start):
                nc.tensor.matmul(pst[:, ds(half * 512, 512)],
                                 qp[j][:, ts(t, 128)],
                                 kp[j][:, ds(half * 512, 512)],
                                 start=start, stop=False)

            def dr_mm(pst, a, t, half, stop):
                nc.tensor.matmul(pst[:, ds(half * 512, 512)],
                                 qp8[a][:, :, ts(t, 128)],
                                 kp8[a][:, :, ds(half * 512, 512)],
                                 start=False, stop=stop, perf_mode=DR)

            def mask_mm(pst, t, half):
                nc.tensor.matmul(pst[:, ds(half * 512, 512)], eye,
                                 mk[t][:, ds(half * 512, 512)],
                                 start=False, stop=False)

            def s01(j, t_sel=(0, 1)):
                for t in t_sel:
                    a = acc0 if t == 0 else acc1
                    for half in range(2):
                        f16_mm(a, j, t, half, j == 0)

            # ---- era: stream = tk0ab,tq0,tk1,tq1,tk2,tq2,tk3,tq3,tk4..tk7.
            # Emission never queues a kproj behind a score mm that waits on
            # a LATER tanh/DVE product.
            kproj0()
            qpair(0)
            kproj(1)
            qpair(1)
            s01(0, (0,))
            kproj(2)
            s01(0, (1,))
            s01(1, (0,))
            qpair(2)
            s01(1, (1,))
            kproj(3)
            s01(2, (0,))
            qpair(3)
            s01(2, (1,))
            kproj(4)
            s01(3, (0,))
            kproj(5)
            s01(3, (1,))
            kproj(6)
            kproj(7)
            # DR pair A for t0/t1 (ready with tk5), mask01 while tk6/7 run
            for t, a in ((0, acc0), (1, acc1)):
                for half in range(2):
                    dr_mm(a, 0, t, half, False)
            for t, a in ((0, acc0), (1, acc1)):
                for half in range(2):
                    mask_mm(a, t, half)

            # ---- post-stream
            p2 = rotp.tile([128, S], F32, tag="rot", name="p2")
            p3 = rotp.tile([128, S], F32, tag="rot", name="p3")

            # t2 f16 chunks 0,1 in slot A (free after tk6-read)
            for half in range(2):
                f16_mm(p2, 0, 2, half, True)
                f16_mm(p2, 1, 2, half, False)
            # t0/t1 stops ASAP after tk7
            for t, a in ((0, acc0), (1, acc1)):
                for half in range(2):
                    dr_mm(a, 1, t, half, True)
            # interleave t2 and t3 (t3 half0 first so its exp can split)
            for half in range(2):
                f16_mm(p2, 2, 2, half, False)
                f16_mm(p2, 3, 2, half, False)
            f16_mm(p3, 0, 3, 0, True)
            mask_mm(p3, 3, 0)
            f16_mm(p3, 1, 3, 0, False)
            for half in range(2):
                dr_mm(p2, 0, 2, half, False)
            for half in range(2):
                dr_mm(p2, 1, 2, half, True)
            f16_mm(p3, 2, 3, 0, False)
            f16_mm(p3, 3, 3, 0, False)
            dr_mm(p3, 0, 3, 0, False)
            dr_mm(p3, 1, 3, 0, True)
            f16_mm(p3, 0, 3, 1, True)
            mask_mm(p3, 3, 1)
            for j in range(1, 4):
                f16_mm(p3, j, 3, 1, False)
            dr_mm(p3, 0, 3, 1, False)
            dr_mm(p3, 1, 3, 1, True)

            # ---- exp (+accum row sums) -> normalize -> out, per block
            ex = [softp.tile([128, S], F16, tag=f"ex{t}", name=f"ex{t}")
                  for t in range(4)]

            def finish(t, src, mul_eng):
                ssum = statp.tile([128, 1], F32, tag="ssum", name=f"ssum{t}")
                nc.scalar.activation(ex[t][:], src, Exp, accum_out=ssum[:])
                rec = statp.tile([128, 1], F32, tag="rec", name=f"rec{t}")
                nc.vector.reciprocal(rec[:], ssum[:])
                ot = obp.tile([128, S], F16, tag="ot", name=f"ot{t}")
                mul_eng.tensor_scalar_mul(ot[:], ex[t][:], rec[:])
                nc.sync.dma_start(out=out[ts(t, 128), :], in_=ot[:])

            finish(0, acc0[:], nc.vector)
            finish(1, acc1[:], nc.vector)
            # t2: unmasked exp; mask (0/1) + row-sum fused in one DVE stt
            exr2 = softp.tile([128, S], F16, tag="exr2", name="exr2")
            s2 = statp.tile([128, 1], F32, tag="ssum", name="ssum2")
            nc.scalar.activation(exr2[:], p2[:], Exp)
            nc.vector.scalar_tensor_tensor(
                ex[2][:], exr2[:], 1.0, mk[2][:],
                op0=mybir.AluOpType.bypass, op1=mybir.AluOpType.mult,
                accum_out=s2[:])
            rec2 = statp.tile([128, 1], F32, tag="rec", name="rec2")
            nc.vector.reciprocal(rec2[:], s2[:])
            ot2 = obp.tile([128, S], F16, tag="ot", name="ot2")
            nc.vector.tensor_scalar_mul(ot2[:], ex[2][:], rec2[:])
            nc.sync.dma_start(out=out[ts(2, 128), :], in_=ot2[:])
            finish(3, p3[:], nc.vector)

    nc.compile()
    return nc


_NC = None


def _get_nc():
    global _NC
    if _NC is None:
        _NC = build_nc()
    return _NC


def make_in_maps(query, key, mask, Wq, bq, Wk, bk, Wc, bc):
    query = np.asarray(query, np.float32)
    key = np.asarray(key, np.float32)
    mask = np.asarray(mask)
    Wq = np.asarray(Wq, np.float32)
    Wk = np.asarray(Wk, np.float32)
    Wc = np.asarray(Wc, np.float32)
    bq = np.asarray(bq, np.float32)
    bk = np.asarray(bk, np.float32)

    # sort heads by |Wc| descending: chunks 0-3 (8 largest) stay fp16,
    # chunks 4-7 (8 smallest) run fp8 DoubleRow
    order = np.argsort(-np.abs(Wc[0]))
    wc_s = Wc[0][order]

    def blockdiag(W):
        blk = np.zeros((128, 128), np.float32)
        blk[0:64, 0:64] = W.T
        blk[64:128, 64:128] = W.T
        return blk

    wk128 = blockdiag(Wk).astype(np.float16)
    wq128 = blockdiag(Wq).astype(np.float16)
    eye = np.eye(128, dtype=np.float16)
    wb16 = np.zeros((128, 10), np.float16)
    wb16[:, 0] = np.tile(bk.reshape(-1), 2)
    wb16[:, 1] = np.tile(bq.reshape(-1), 2)
    for j in range(NJ):
        wb16[0:64, 2 + j] = wc_s[2 * j]
        wb16[64:128, 2 + j] = wc_s[2 * j + 1]

    in_maps = []
    for c in range(NCORES):
        b, half = divmod(c, 2)
        s0 = half * SQ
        qh = query[b].reshape(H, S, DK)[order][:, s0:s0 + SQ, :]
        qTc = np.ascontiguousarray(
            qh.transpose(0, 2, 1)).reshape(NJ, 128, SQ).astype(np.float16)
        kh_ = key[b].reshape(H, S, DK)[order]
        kTc = np.ascontiguousarray(
            kh_.transpose(0, 2, 1)).reshape(NJ, 128, S).astype(np.float16)
        pre = np.concatenate(
            [wk128, wb16, kTc[0][:, 0:512], wq128, eye, kTc[0][:, 512:1024],
             qTc[0], qTc[1], kTc[1], qTc[2], qTc[3], kTc[2],
             qTc[4], qTc[5], qTc[6], qTc[7]], axis=1)
        assert pre.shape[1] == PRE_COLS, pre.shape
        mc = np.where(mask[b, s0:s0 + SQ, :] == 0, NEGM, 0.0)
        mc = np.ascontiguousarray(mc.reshape(4, 128, S)).astype(np.float16)
        mc[2] = np.ascontiguousarray(
            mask[b, s0 + 256:s0 + 384, :]).astype(np.float16)
        in_maps.append({"pre": pre, "kT": kTc[3:], "msk": mc})
    return in_maps


def kernel(query, key, mask, Wq, bq, Wk, bk, Wc, bc):
    from concourse.bass_utils import run_bass_kernel_spmd

    nc = _get_nc()
    in_maps = make_in_maps(query, key, mask, Wq, bq, Wk, bk, Wc, bc)
    res = run_bass_kernel_spmd(nc, in_maps, list(range(NCORES)))
    full = np.empty((B, S, S), np.float32)
    for c in range(NCORES):
        b, half = divmod(c, 2)
        full[b, half * SQ:(half + 1) * SQ, :] = \
            res.results[c]["out"].astype(np.float32)
    return full
